# revision 1
# baseline (speedup 1.0000x reference)
# Trainium2 Bass kernel for nn_MultiHeadAttentionPure (B=2, S=1024, F=1024, H=16).
#
# The reference splits q/k/v into 64 feature-chunks of 16 ("groups"), runs
# causal attention independently per (group, batch) pair -- 128 independent
# [1024,16] attention problems -- then applies a (buggy-but-faithful) torch
# reshape that scrambles (group, batch, seq) into the [B,S,F] tensor fed to
# the output linear layer.
#
# Sharding: the scramble maps output rows (b2, s2) to attention groups such
# that core c = b2*4 + q (q = s2_block of 256) needs exactly the 16 groups
# {j : j%4 == 2*b2 + q//2} at input batch b = q%2 -- a perfect partition of
# the 128 (group, batch) pairs across 8 cores with zero cross-core traffic.
# Each core computes its 16 attention groups, assembles its y^T tile
# ([1024 features, 256 rows]) on-chip, and runs the output linear for its
# 256 output rows.  Host slices inputs / concatenates outputs.
#
# On-device layout (per core, per group g):
#   scores^T[s2, s1] = k^T-slice (lhsT [16,128]) x q^T (rhs [16,512])  (fp32r)
#   causal mask: DVE adds -1e9 triangle onto PSUM before exp
#   expT = ACT Exp(PSUM) -> SBUF fp32r
#   x^T [33, s1] += vaug (lhsT [128,33], col 32 = ones) x expT   => row 32 = denom
#   recip = 1/denom (fp32r), PE-broadcast to 16 partitions via ones[1,16]
#   xs[h,m,r] = x^T[h, 4r+m] * recip  (DVE), DMA-scatter into y^T tile
#   out[r, o] = sum_f y^T[f,r] x W_out^T[f,o] + b_out  (fp32r matmuls)
import numpy as np

B, S, F, H = 2, 1024, 1024, 16
NG = 16          # groups per core
P = 128
NCORES = 8


def _fp32r(x):
    """Round fp32 -> fp32r (11-bit mantissa, round-half-up) like the HW expects."""
    b = np.ascontiguousarray(x.astype(np.float32)).view(np.uint32)
    r = ((b.astype(np.uint64) + 0x800) & 0xFFFFF000).astype(np.uint32)
    return r.view(np.float32)


def _core_groups(c):
    b2, qq = c // 4, c % 4
    b = qq % 2
    jmod = 2 * b2 + qq // 2
    js = [4 * h2 + jmod for h2 in range(NG)]
    return b2, qq, b, js


def _build(causal: bool):
    import concourse.bass as bass
    import concourse.mybir as mybir
    from concourse import bacc, tile

    F32 = mybir.dt.float32
    F32R = mybir.dt.float32r
    AF = mybir.ActivationFunctionType
    ADD = mybir.AluOpType.add
    MUL = mybir.AluOpType.mult

    nc = bacc.Bacc("TRN2", target_bir_lowering=False, debug=False)
    qt = nc.declare_dram_parameter("qt", [NG * H, S], F32R, isOutput=False)
    kt = nc.declare_dram_parameter("kt", [NG * H, S], F32R, isOutput=False)
    va = nc.declare_dram_parameter("va", [S, NG * 33], F32R, isOutput=False)
    wt = nc.declare_dram_parameter("wt", [F, F], F32R, isOutput=False)
    ones = nc.declare_dram_parameter("ones", [1, 16], F32R, isOutput=False)
    msk = nc.declare_dram_parameter("msk", [P, 256], F32, isOutput=False)
    bb = nc.declare_dram_parameter("bb", [P, F], F32, isOutput=False)
    out = nc.declare_dram_parameter("o", [256, F], F32, isOutput=True)

    NT = S // P           # 8 s2 tiles
    NC_ = S // 512        # 2 s1 chunks

    with tile.TileContext(nc) as tc:
        with tc.tile_pool(name="cst", bufs=1) as cst, \
             tc.tile_pool(name="qk", bufs=3) as qkp, \
             tc.tile_pool(name="expp", bufs=2) as expp, \
             tc.tile_pool(name="sc", bufs=2) as scp, \
             tc.tile_pool(name="work", bufs=2) as wkp, \
             tc.tile_pool(name="yt", bufs=1) as ytp, \
             tc.tile_pool(name="stps", bufs=2, space="PSUM") as stps, \
             tc.tile_pool(name="xtps", bufs=2, space="PSUM") as xtps, \
             tc.tile_pool(name="rbps", bufs=2, space="PSUM") as rbps:

            va_sb = cst.tile([P, NT, NG * 33], F32R)
            wt_sb = cst.tile([P, F // P, F], F32R)
            ones_sb = cst.tile([1, 16], F32R)
            msk_sb = cst.tile([P, 256], F32)
            bb_sb = cst.tile([P, F], F32)
            nc.sync.dma_start(va_sb[:], va.rearrange("(t p) m -> p t m", p=P))
            nc.sync.dma_start(wt_sb[:], wt.rearrange("(t p) m -> p t m", p=P))
            nc.sync.dma_start(ones_sb[:], ones[:])
            nc.sync.dma_start(msk_sb[:], msk[:])
            nc.sync.dma_start(bb_sb[:], bb[:])

            yt_sb = ytp.tile([P, F // P, 256], F32R)

            for g in range(NG):
                qt_g = qkp.tile([H, S], F32R, tag="qt")
                kt_g = qkp.tile([H, S], F32R, tag="kt")
                nc.sync.dma_start(qt_g[:], qt[g * H:(g + 1) * H, :])
                nc.sync.dma_start(kt_g[:], kt[g * H:(g + 1) * H, :])
                for c in range(NC_):
                    ntile = 4 * c + 4 if causal else NT
                    expt = expp.tile([P, NT, 512], F32R, tag="expt")
                    for t in range(ntile):
                        st = stps.tile([P, 512], F32, tag="st")
                        d = t - 4 * c
                        if causal and 0 <= d:
                            a1 = 128 * d
                            a0 = min(a1, 256)
                            w = a1 - a0 + 128
                        else:
                            a1 = a0 = 0
                            w = 0
                        nc.tensor.matmul(
                            st[:, a0:], kt_g[:, t * P:(t + 1) * P],
                            qt_g[:, 512 * c + a0: 512 * (c + 1)],
                            start=True, stop=True)
                        if w:
                            nc.vector.tensor_tensor(
                                out=st[:, a0:a1 + P], in0=st[:, a0:a1 + P],
                                in1=msk_sb[:, 256 - w:], op=ADD)
                        nc.scalar.activation(expt[:, t, a0:], st[:, a0:], AF.Exp)
                    xt = xtps.tile([33, 512], F32, tag="xt")
                    for t in range(ntile):
                        a0 = min(max(0, 128 * (t - 4 * c)), 256) if causal else 0
                        nc.tensor.matmul(
                            xt[:, a0:], va_sb[:, t, g * 33:(g + 1) * 33],
                            expt[:, t, a0:],
                            start=(t == 0), stop=(t == ntile - 1))
                    recip = wkp.tile([1, 512], F32R, tag="recip")
                    with nc.allow_low_precision(reason="fp32r recip for PE bcast"):
                        nc.vector.reciprocal(recip[:], xt[32:33, :])
                    rb = rbps.tile([16, 512], F32, tag="rb")
                    nc.tensor.matmul(rb[:], ones_sb[:], recip[:], start=True, stop=True)
                    recipb = wkp.tile([16, 512], F32, tag="recipb")
                    nc.vector.tensor_copy(recipb[:], rb[:])
                    xs = wkp.tile([16, 4, 128], F32R, tag="xs")
                    for m in range(4):
                        nc.vector.tensor_tensor(
                            out=xs[:, m, :], in0=xt[0:16, m:512:4],
                            in1=recipb[:, m:512:4], op=MUL)
                    po = 64 * (g % 2)
                    for m in range(4):
                        nc.sync.dma_start(
                            out=yt_sb[po + 16 * m: po + 16 * (m + 1), g // 2,
                                      128 * c:128 * (c + 1)],
                            in_=xs[:, m, :])

            # output linear: out[r, o] = sum_f yT[f, r] * wt[f, o] + b[o]
            for r2 in range(2):
                for oc in range(2):
                    ps = stps.tile([P, 512], F32, tag="st")
                    for ft in range(F // P):
                        nc.tensor.matmul(
                            ps[:], yt_sb[:, ft, r2 * P:(r2 + 1) * P],
                            wt_sb[:, ft, oc * 512:(oc + 1) * 512],
                            start=(ft == 0), stop=(ft == F // P - 1))
                    ot = wkp.tile([P, 512], F32, tag="ot")
                    nc.vector.tensor_tensor(
                        out=ot[:], in0=ps[:], in1=bb_sb[:, oc * 512:(oc + 1) * 512],
                        op=ADD)
                    nc.sync.dma_start(
                        out=out[r2 * P:(r2 + 1) * P, oc * 512:(oc + 1) * 512],
                        in_=ot[:])
    nc.compile()
    return nc


_NC_CACHE = {}


def _get_nc(causal: bool):
    if causal not in _NC_CACHE:
        _NC_CACHE[causal] = _build(causal)
    return _NC_CACHE[causal]


def _shard_inputs(q, k, v, W_out, b_out):
    """Build the 8 per-core input maps (all fp32r pre-rounded where needed)."""
    wt = _fp32r(np.ascontiguousarray(W_out.T))
    ones16 = np.ones((1, 16), np.float32)
    mskv = np.full((P, 256), -1e9, np.float32)
    xi, yi = np.mgrid[0:P, 0:P]
    mskv[:, 128:] = np.where(yi >= xi, 0.0, -1e9).astype(np.float32)
    bbv = np.broadcast_to(b_out.astype(np.float32), (P, F)).copy()

    in_maps = []
    for c in range(NCORES):
        _, _, b, js = _core_groups(c)
        cols = np.concatenate([j * H + np.arange(H) for j in js])
        qc = _fp32r(0.25 * q[b][:, cols].T)          # [256, S]
        kc = _fp32r(np.ascontiguousarray(k[b][:, cols].T))
        vav = np.zeros((S, NG, 33), np.float32)
        vav[:, :, :16] = v[b][:, cols].reshape(S, NG, H)
        vav[:, :, 32] = 1.0
        in_maps.append({
            "qt": np.ascontiguousarray(qc),
            "kt": kc,
            "va": _fp32r(vav.reshape(S, NG * 33)),
            "wt": wt,
            "ones": ones16,
            "msk": mskv,
            "bb": bbv,
        })
    return in_maps


def _unshard(outs):
    full = np.empty((B, S, F), np.float32)
    for c in range(NCORES):
        b2, qq, _, _ = _core_groups(c)
        full[b2, 256 * qq:256 * (qq + 1), :] = outs[c]
    return full


def _numpy_core(in_map, causal=True):
    """Numpy emulation of the device program (for host-logic validation)."""
    qt = in_map["qt"]; kt = in_map["kt"]
    va = in_map["va"].reshape(S, NG, 33)
    wtm = in_map["wt"]; bbv = in_map["bb"]
    ytv = np.zeros((F, 256), np.float32)
    for g in range(NG):
        sc = kt[g * H:(g + 1) * H].T @ qt[g * H:(g + 1) * H]   # [s2, s1]
        if causal:
            s2i, s1i = np.mgrid[0:S, 0:S]
            sc = np.where(s1i >= s2i, sc, -1e9)
        e = _fp32r(np.exp(sc).astype(np.float32))
        if causal:
            e = np.where(s1i >= s2i, e, 0.0).astype(np.float32)
        xt = va[:, g, :].T @ e                                  # [33, s1]
        recip = _fp32r(1.0 / xt[32])
        xs = _fp32r(xt[0:16] * recip[None, :])                  # [h, s1]
        po = 64 * (g % 2)
        for m in range(4):
            for cc in range(2):
                ytv[128 * (g // 2) + po + 16 * m: 128 * (g // 2) + po + 16 * (m + 1),
                    128 * cc:128 * (cc + 1)] = xs[:, 512 * cc + m:512 * (cc + 1):4]
    o = ytv.T @ wtm + bbv[0][None, :]
    return o.astype(np.float32)


def kernel(q, k, v, W_out, b_out, apply_mask, _mock=False):
    q = np.asarray(q, np.float32)
    k = np.asarray(k, np.float32)
    v = np.asarray(v, np.float32)
    W_out = np.asarray(W_out, np.float32)
    b_out = np.asarray(b_out, np.float32)
    causal = bool(int(np.asarray(apply_mask)))
    in_maps = _shard_inputs(q, k, v, W_out, b_out)
    if _mock:
        outs = [_numpy_core(m, causal) for m in in_maps]
        return _unshard(outs)
    from concourse.bass_utils import run_bass_kernel_spmd
    nc = _get_nc(causal)
    res = run_bass_kernel_spmd(nc, in_maps, core_ids=list(range(NCORES)))
    return _unshard([r["o"] for r in res.results])


# revision 2
# speedup vs baseline: 6581.6115x; 6581.6115x over previous
# Trainium2 Bass kernel for nn_MultiHeadAttentionPure (B=2, S=1024, F=1024, H=16).
#
# The reference splits q/k/v into 64 feature-chunks of 16 ("groups"), runs
# causal attention independently per (group, batch) pair -- 128 independent
# [1024,16] attention problems -- then applies a (buggy-but-faithful) torch
# reshape that scrambles (group, batch, seq) into the [B,S,F] tensor fed to
# the output linear layer.
#
# Sharding: the scramble maps output rows (b2, s2) to attention groups such
# that core c = b2*4 + q (q = s2_block of 256) needs exactly the 16 groups
# {j : j%4 == 2*b2 + q//2} at input batch b = q%2 -- a perfect partition of
# the 128 (group, batch) pairs across 8 cores with zero cross-core traffic.
# Each core computes its 16 attention groups, assembles its y^T tile
# ([1024 features, 256 rows]) on-chip, and runs the output linear for its
# 256 output rows.  Host slices inputs / concatenates outputs.
#
# On-device layout (per core, per group g):
#   scores^T[s2, s1] = k^T-slice (lhsT [16,128]) x q^T (rhs [16,512])  (fp32r)
#   causal mask: DVE adds -1e9 triangle onto PSUM before exp
#   expT = ACT Exp(PSUM) -> SBUF fp32r
#   x^T [33, s1] += vaug (lhsT [128,33], col 32 = ones) x expT   => row 32 = denom
#   recip = 1/denom (fp32r), PE-broadcast to 16 partitions via ones[1,16]
#   xs[h,m,r] = x^T[h, 4r+m] * recip  (DVE), DMA-scatter into y^T tile
#   out[r, o] = sum_f y^T[f,r] x W_out^T[f,o] + b_out  (fp32r matmuls)
import numpy as np

B, S, F, H = 2, 1024, 1024, 16
NG = 16          # groups per core
P = 128
NCORES = 8


def _fp32r(x):
    """Round fp32 -> fp32r (11-bit mantissa, round-half-up) like the HW expects."""
    b = np.ascontiguousarray(x.astype(np.float32)).view(np.uint32)
    r = ((b.astype(np.uint64) + 0x800) & 0xFFFFF000).astype(np.uint32)
    return r.view(np.float32)


def _core_groups(c):
    b2, qq = c // 4, c % 4
    b = qq % 2
    jmod = 2 * b2 + qq // 2
    js = [4 * h2 + jmod for h2 in range(NG)]
    return b2, qq, b, js


def _build(causal: bool, n_iter: int = 1):
    import concourse.bass as bass
    import concourse.mybir as mybir
    from concourse import bacc, tile

    F32 = mybir.dt.float32
    F32R = mybir.dt.float32r
    AF = mybir.ActivationFunctionType
    ADD = mybir.AluOpType.add
    MUL = mybir.AluOpType.mult

    nc = bacc.Bacc("TRN2", target_bir_lowering=False, debug=False)
    qt = nc.declare_dram_parameter("qt", [NG * H, S], F32R, isOutput=False)
    kt = nc.declare_dram_parameter("kt", [NG * H, S], F32R, isOutput=False)
    va = nc.declare_dram_parameter("va", [S, NG * 33], F32R, isOutput=False)
    wt = nc.declare_dram_parameter("wt", [F, F], F32R, isOutput=False)
    ones = nc.declare_dram_parameter("ones", [1, 16], F32R, isOutput=False)
    msk = nc.declare_dram_parameter("msk", [P, 256], F32, isOutput=False)
    bb = nc.declare_dram_parameter("bb", [P, F], F32, isOutput=False)
    out = nc.declare_dram_parameter("o", [256, F], F32, isOutput=True)

    NT = S // P           # 8 s2 tiles
    NC_ = S // 512        # 2 s1 chunks

    import contextlib
    with tile.TileContext(nc) as tc:
        loop_ctx = tc.For_i(0, n_iter, 1) if n_iter > 1 else contextlib.nullcontext()
        with loop_ctx, \
             tc.tile_pool(name="cst", bufs=1) as cst, \
             tc.tile_pool(name="qk", bufs=3) as qkp, \
             tc.tile_pool(name="expp", bufs=2) as expp, \
             tc.tile_pool(name="sc", bufs=2) as scp, \
             tc.tile_pool(name="work", bufs=2) as wkp, \
             tc.tile_pool(name="yt", bufs=1) as ytp, \
             tc.tile_pool(name="stps", bufs=2, space="PSUM") as stps, \
             tc.tile_pool(name="xtps", bufs=2, space="PSUM") as xtps, \
             tc.tile_pool(name="rbps", bufs=2, space="PSUM") as rbps:

            va_sb = cst.tile([P, NT, NG * 33], F32R)
            wt_sb = cst.tile([P, F // P, F], F32R)
            ones_sb = cst.tile([1, 16], F32R)
            msk_sb = cst.tile([P, 256], F32)
            bb_sb = cst.tile([P, F], F32)
            nc.sync.dma_start(va_sb[:], va.rearrange("(t p) m -> p t m", p=P))
            nc.sync.dma_start(wt_sb[:], wt.rearrange("(t p) m -> p t m", p=P))
            nc.sync.dma_start(ones_sb[:], ones[:])
            nc.sync.dma_start(msk_sb[:], msk[:])
            nc.sync.dma_start(bb_sb[:], bb[:])

            yt_sb = ytp.tile([P, F // P, 256], F32R)

            for g in range(NG):
                qt_g = qkp.tile([H, S], F32R, tag="qt")
                kt_g = qkp.tile([H, S], F32R, tag="kt")
                nc.sync.dma_start(qt_g[:], qt[g * H:(g + 1) * H, :])
                nc.sync.dma_start(kt_g[:], kt[g * H:(g + 1) * H, :])
                for c in range(NC_):
                    ntile = 4 * c + 4 if causal else NT
                    expt = expp.tile([P, NT, 512], F32R, tag="expt")
                    for t in range(ntile):
                        st = stps.tile([P, 512], F32, tag="st")
                        d = t - 4 * c
                        if causal and 0 <= d:
                            a1 = 128 * d
                            a0 = min(a1, 256)
                            w = a1 - a0 + 128
                        else:
                            a1 = a0 = 0
                            w = 0
                        nc.tensor.matmul(
                            st[:, a0:], kt_g[:, t * P:(t + 1) * P],
                            qt_g[:, 512 * c + a0: 512 * (c + 1)],
                            start=True, stop=True)
                        if w:
                            nc.vector.tensor_tensor(
                                out=st[:, a0:a1 + P], in0=st[:, a0:a1 + P],
                                in1=msk_sb[:, 256 - w:], op=ADD)
                        nc.scalar.activation(expt[:, t, a0:], st[:, a0:], AF.Exp)
                    xt = xtps.tile([33, 512], F32, tag="xt")
                    for t in range(ntile):
                        a0 = min(max(0, 128 * (t - 4 * c)), 256) if causal else 0
                        nc.tensor.matmul(
                            xt[:, a0:], va_sb[:, t, g * 33:(g + 1) * 33],
                            expt[:, t, a0:],
                            start=(t == 0), stop=(t == ntile - 1))
                    recip = wkp.tile([1, 512], F32R, tag="recip")
                    with nc.allow_low_precision(reason="fp32r recip for PE bcast"):
                        nc.vector.reciprocal(recip[:], xt[32:33, :])
                    rb = rbps.tile([16, 512], F32, tag="rb")
                    nc.tensor.matmul(rb[:], ones_sb[:], recip[:], start=True, stop=True)
                    recipb = wkp.tile([16, 512], F32, tag="recipb")
                    nc.vector.tensor_copy(recipb[:], rb[:])
                    xs = wkp.tile([16, 4, 128], F32R, tag="xs")
                    for m in range(4):
                        nc.vector.tensor_tensor(
                            out=xs[:, m, :], in0=xt[0:16, m:512:4],
                            in1=recipb[:, m:512:4], op=MUL)
                    po = 64 * (g % 2)
                    for m in range(4):
                        nc.sync.dma_start(
                            out=yt_sb[po + 16 * m: po + 16 * (m + 1), g // 2,
                                      128 * c:128 * (c + 1)],
                            in_=xs[:, m, :])

            # output linear: out[r, o] = sum_f yT[f, r] * wt[f, o] + b[o]
            for r2 in range(2):
                for oc in range(2):
                    ps = stps.tile([P, 512], F32, tag="st")
                    for ft in range(F // P):
                        nc.tensor.matmul(
                            ps[:], yt_sb[:, ft, r2 * P:(r2 + 1) * P],
                            wt_sb[:, ft, oc * 512:(oc + 1) * 512],
                            start=(ft == 0), stop=(ft == F // P - 1))
                    ot = wkp.tile([P, 512], F32, tag="ot")
                    nc.vector.tensor_tensor(
                        out=ot[:], in0=ps[:], in1=bb_sb[:, oc * 512:(oc + 1) * 512],
                        op=ADD)
                    nc.sync.dma_start(
                        out=out[r2 * P:(r2 + 1) * P, oc * 512:(oc + 1) * 512],
                        in_=ot[:])
    nc.compile()
    return nc


_NC_CACHE = {}


def _get_nc(causal: bool, n_iter: int = 1):
    key = (causal, n_iter)
    if key not in _NC_CACHE:
        _NC_CACHE[key] = _build(causal, n_iter)
    return _NC_CACHE[key]


def _shard_inputs(q, k, v, W_out, b_out):
    """Build the 8 per-core input maps (all fp32r pre-rounded where needed)."""
    wt = _fp32r(np.ascontiguousarray(W_out.T))
    ones16 = np.ones((1, 16), np.float32)
    mskv = np.full((P, 256), -1e9, np.float32)
    xi, yi = np.mgrid[0:P, 0:P]
    mskv[:, 128:] = np.where(yi >= xi, 0.0, -1e9).astype(np.float32)
    bbv = np.broadcast_to(b_out.astype(np.float32), (P, F)).copy()

    in_maps = []
    for c in range(NCORES):
        _, _, b, js = _core_groups(c)
        cols = np.concatenate([j * H + np.arange(H) for j in js])
        qc = _fp32r(0.25 * q[b][:, cols].T)          # [256, S]
        kc = _fp32r(np.ascontiguousarray(k[b][:, cols].T))
        vav = np.zeros((S, NG, 33), np.float32)
        vav[:, :, :16] = v[b][:, cols].reshape(S, NG, H)
        vav[:, :, 32] = 1.0
        in_maps.append({
            "qt": np.ascontiguousarray(qc),
            "kt": kc,
            "va": _fp32r(vav.reshape(S, NG * 33)),
            "wt": wt,
            "ones": ones16,
            "msk": mskv,
            "bb": bbv,
        })
    return in_maps


def _unshard(outs):
    full = np.empty((B, S, F), np.float32)
    for c in range(NCORES):
        b2, qq, _, _ = _core_groups(c)
        full[b2, 256 * qq:256 * (qq + 1), :] = outs[c]
    return full


def _numpy_core(in_map, causal=True):
    """Numpy emulation of the device program (for host-logic validation)."""
    qt = in_map["qt"]; kt = in_map["kt"]
    va = in_map["va"].reshape(S, NG, 33)
    wtm = in_map["wt"]; bbv = in_map["bb"]
    ytv = np.zeros((F, 256), np.float32)
    for g in range(NG):
        sc = kt[g * H:(g + 1) * H].T @ qt[g * H:(g + 1) * H]   # [s2, s1]
        if causal:
            s2i, s1i = np.mgrid[0:S, 0:S]
            sc = np.where(s1i >= s2i, sc, -1e9)
        e = _fp32r(np.exp(sc).astype(np.float32))
        if causal:
            e = np.where(s1i >= s2i, e, 0.0).astype(np.float32)
        xt = va[:, g, :].T @ e                                  # [33, s1]
        recip = _fp32r(1.0 / xt[32])
        xs = _fp32r(xt[0:16] * recip[None, :])                  # [h, s1]
        po = 64 * (g % 2)
        for m in range(4):
            for cc in range(2):
                ytv[128 * (g // 2) + po + 16 * m: 128 * (g // 2) + po + 16 * (m + 1),
                    128 * cc:128 * (cc + 1)] = xs[:, 512 * cc + m:512 * (cc + 1):4]
    o = ytv.T @ wtm + bbv[0][None, :]
    return o.astype(np.float32)


def kernel(q, k, v, W_out, b_out, apply_mask, _mock=False):
    q = np.asarray(q, np.float32)
    k = np.asarray(k, np.float32)
    v = np.asarray(v, np.float32)
    W_out = np.asarray(W_out, np.float32)
    b_out = np.asarray(b_out, np.float32)
    causal = bool(int(np.asarray(apply_mask)))
    in_maps = _shard_inputs(q, k, v, W_out, b_out)
    if _mock:
        outs = [_numpy_core(m, causal) for m in in_maps]
        return _unshard(outs)
    from concourse.bass_utils import run_bass_kernel_spmd
    nc = _get_nc(causal)
    res = run_bass_kernel_spmd(nc, in_maps, core_ids=list(range(NCORES)))
    return _unshard([r["o"] for r in res.results])


# revision 4
# speedup vs baseline: 17378.5647x; 2.6405x over previous
# Trainium2 Bass kernel for nn_MultiHeadAttentionPure (B=2, S=1024, F=1024, H=16).
#
# The reference splits q/k/v into 64 feature-chunks of 16 ("groups"), runs
# causal attention independently per (group, batch) pair -- 128 independent
# [1024,16] attention problems -- then applies a (buggy-but-faithful) torch
# reshape that scrambles (group, batch, seq) into the [B,S,F] tensor fed to
# the output linear layer.
#
# Sharding: the scramble maps output rows (b2, s2) to attention groups such
# that core c = b2*4 + q (q = s2_block of 256) needs exactly the 16 groups
# {j : j%4 == 2*b2 + q//2} at input batch b = q%2 -- a perfect partition of
# the 128 (group, batch) pairs across 8 cores with zero cross-core traffic.
# Each core computes its 16 attention groups, assembles its y^T tile
# ([1024 features, 256 rows]) on-chip, and runs the output linear for its
# 256 output rows.  Host slices inputs / concatenates outputs.
#
# On-device layout (per core, per group g):
#   scores^T[s2, s1] = k^T-slice (lhsT [16,128]) x q^T (rhs [16,512])  (fp32r)
#   causal mask: DVE adds -1e9 triangle onto PSUM before exp
#   expT = ACT Exp(PSUM) -> SBUF fp32r
#   x^T [33, s1] += vaug (lhsT [128,33], col 32 = ones) x expT   => row 32 = denom
#   recip = 1/denom (fp32r), PE-broadcast to 16 partitions via ones[1,16]
#   xs[h,m,r] = x^T[h, 4r+m] * recip  (DVE), DMA-scatter into y^T tile
#   out[r, o] = sum_f y^T[f,r] x W_out^T[f,o] + b_out  (fp32r matmuls)
import numpy as np

B, S, F, H = 2, 1024, 1024, 16
NG = 16          # groups per core
P = 128
NCORES = 8


def _fp32r(x):
    """Round fp32 -> fp32r (11-bit mantissa, round-half-up) like the HW expects."""
    b = np.ascontiguousarray(x.astype(np.float32)).view(np.uint32)
    r = ((b.astype(np.uint64) + 0x800) & 0xFFFFF000).astype(np.uint32)
    return r.view(np.float32)


def _core_groups(c):
    b2, qq = c // 4, c % 4
    b = qq % 2
    jmod = 2 * b2 + qq // 2
    js = [4 * h2 + jmod for h2 in range(NG)]
    return b2, qq, b, js


def _build(causal: bool, n_iter: int = 1):
    import concourse.bass as bass
    import concourse.mybir as mybir
    from concourse import bacc, tile

    F32 = mybir.dt.float32
    F32R = mybir.dt.float16   # attention/linear operand dtype (full-rate PE)
    AF = mybir.ActivationFunctionType
    ADD = mybir.AluOpType.add
    MUL = mybir.AluOpType.mult

    nc = bacc.Bacc("TRN2", target_bir_lowering=False, debug=False)
    qt = nc.declare_dram_parameter("qt", [NG * H, S], F32R, isOutput=False)
    kt = nc.declare_dram_parameter("kt", [NG * H, S], F32R, isOutput=False)
    va = nc.declare_dram_parameter("va", [S, NG * 33], F32R, isOutput=False)
    wt = nc.declare_dram_parameter("wt", [F, F], F32R, isOutput=False)
    ones = nc.declare_dram_parameter("ones", [1, 16], F32R, isOutput=False)
    msk = nc.declare_dram_parameter("msk", [P, 256], F32, isOutput=False)
    bb = nc.declare_dram_parameter("bb", [P, F], F32, isOutput=False)
    out = nc.declare_dram_parameter("o", [256, F], F32, isOutput=True)

    NT = S // P           # 8 s2 tiles
    NC_ = S // 512        # 2 s1 chunks

    import contextlib
    with tile.TileContext(nc) as tc:
        loop_ctx = tc.For_i(0, n_iter, 1) if n_iter > 1 else contextlib.nullcontext()
        with loop_ctx, \
             tc.tile_pool(name="cst", bufs=1) as cst, \
             tc.tile_pool(name="qk", bufs=3) as qkp, \
             tc.tile_pool(name="expp", bufs=3) as expp, \
             tc.tile_pool(name="sc", bufs=2) as scp, \
             tc.tile_pool(name="work", bufs=2) as wkp, \
             tc.tile_pool(name="yt", bufs=1) as ytp, \
             tc.tile_pool(name="stps", bufs=4, space="PSUM") as stps, \
             tc.tile_pool(name="xtps", bufs=2, space="PSUM") as xtps, \
             tc.tile_pool(name="rbps", bufs=2, space="PSUM") as rbps:

            va_sb = cst.tile([P, NT, NG * 33], F32R)
            wt_sb = cst.tile([P, F // P, F], F32R)
            ones_sb = cst.tile([1, 16], F32R)
            msk_sb = cst.tile([P, 256], F32)
            bb_sb = cst.tile([P, F], F32)
            nc.sync.dma_start(va_sb[:], va.rearrange("(t p) m -> p t m", p=P))
            nc.sync.dma_start(wt_sb[:], wt.rearrange("(t p) m -> p t m", p=P))
            nc.sync.dma_start(ones_sb[:], ones[:])
            nc.sync.dma_start(msk_sb[:], msk[:])
            nc.sync.dma_start(bb_sb[:], bb[:])

            yt_sb = ytp.tile([P, F // P, 256], F32R)

            for g in range(NG):
                qt_g = qkp.tile([H, S], F32R, tag="qt")
                kt_g = qkp.tile([H, S], F32R, tag="kt")
                nc.sync.dma_start(qt_g[:], qt[g * H:(g + 1) * H, :])
                nc.sync.dma_start(kt_g[:], kt[g * H:(g + 1) * H, :])
                for c in range(NC_):
                    ntile = 4 * c + 4 if causal else NT
                    expt = expp.tile([P, NT, 512], F32R, tag="expt")
                    for t in range(ntile):
                        st = stps.tile([P, 512], F32, tag="st")
                        d = t - 4 * c
                        if causal and 0 <= d:
                            a1 = 128 * d
                            a0 = min(a1, 256)
                            w = a1 - a0 + 128
                        else:
                            a1 = a0 = 0
                            w = 0
                        nc.tensor.matmul(
                            st[:, a0:], kt_g[:, t * P:(t + 1) * P],
                            qt_g[:, 512 * c + a0: 512 * (c + 1)],
                            start=True, stop=True)
                        if w:
                            nc.vector.tensor_tensor(
                                out=st[:, a0:a1 + P], in0=st[:, a0:a1 + P],
                                in1=msk_sb[:, 256 - w:], op=ADD)
                        nc.scalar.activation(expt[:, t, a0:], st[:, a0:], AF.Exp)
                    xt = xtps.tile([33, 512], F32, tag="xt")
                    for t in range(ntile):
                        a0 = min(max(0, 128 * (t - 4 * c)), 256) if causal else 0
                        nc.tensor.matmul(
                            xt[:, a0:], va_sb[:, t, g * 33:(g + 1) * 33],
                            expt[:, t, a0:],
                            start=(t == 0), stop=(t == ntile - 1))
                    recip = wkp.tile([1, 512], F32R, tag="recip")
                    with nc.allow_low_precision(reason="fp32r recip for PE bcast"):
                        nc.vector.reciprocal(recip[:], xt[32:33, :])
                    rb = rbps.tile([16, 512], F32, tag="rb")
                    nc.tensor.matmul(rb[:], ones_sb[:], recip[:], start=True, stop=True)
                    recipb = wkp.tile([16, 512], F32, tag="recipb")
                    nc.vector.tensor_copy(recipb[:], rb[:])
                    xs = wkp.tile([16, 4, 128], F32R, tag="xs")
                    for m in range(4):
                        nc.vector.tensor_tensor(
                            out=xs[:, m, :], in0=xt[0:16, m:512:4],
                            in1=recipb[:, m:512:4], op=MUL)
                    po = 64 * (g % 2)
                    for m in range(4):
                        nc.sync.dma_start(
                            out=yt_sb[po + 16 * m: po + 16 * (m + 1), g // 2,
                                      128 * c:128 * (c + 1)],
                            in_=xs[:, m, :])

            # output linear: out[r, o] = sum_f yT[f, r] * wt[f, o] + b[o]
            for r2 in range(2):
                for oc in range(2):
                    ps = stps.tile([P, 512], F32, tag="st")
                    for ft in range(F // P):
                        nc.tensor.matmul(
                            ps[:], yt_sb[:, ft, r2 * P:(r2 + 1) * P],
                            wt_sb[:, ft, oc * 512:(oc + 1) * 512],
                            start=(ft == 0), stop=(ft == F // P - 1))
                    ot = wkp.tile([P, 512], F32, tag="ot")
                    nc.vector.tensor_tensor(
                        out=ot[:], in0=ps[:], in1=bb_sb[:, oc * 512:(oc + 1) * 512],
                        op=ADD)
                    nc.sync.dma_start(
                        out=out[r2 * P:(r2 + 1) * P, oc * 512:(oc + 1) * 512],
                        in_=ot[:])
    nc.compile()
    return nc


_NC_CACHE = {}


def _get_nc(causal: bool, n_iter: int = 1):
    key = (causal, n_iter)
    if key not in _NC_CACHE:
        _NC_CACHE[key] = _build(causal, n_iter)
    return _NC_CACHE[key]


def _shard_inputs(q, k, v, W_out, b_out):
    """Build the 8 per-core input maps (all fp32r pre-rounded where needed)."""
    wt = np.ascontiguousarray(W_out.T).astype(np.float16)
    ones16 = np.ones((1, 16), np.float16)
    mskv = np.full((P, 256), -1e9, np.float32)
    xi, yi = np.mgrid[0:P, 0:P]
    mskv[:, 128:] = np.where(yi >= xi, 0.0, -1e9).astype(np.float32)
    bbv = np.broadcast_to(b_out.astype(np.float32), (P, F)).copy()

    in_maps = []
    for c in range(NCORES):
        _, _, b, js = _core_groups(c)
        cols = np.concatenate([j * H + np.arange(H) for j in js])
        qc = (0.25 * q[b][:, cols].T).astype(np.float16)     # [256, S]
        kc = np.ascontiguousarray(k[b][:, cols].T).astype(np.float16)
        vav = np.zeros((S, NG, 33), np.float32)
        vav[:, :, :16] = v[b][:, cols].reshape(S, NG, H)
        vav[:, :, 32] = 1.0
        in_maps.append({
            "qt": np.ascontiguousarray(qc),
            "kt": kc,
            "va": vav.reshape(S, NG * 33).astype(np.float16),
            "wt": wt,
            "ones": ones16,
            "msk": mskv,
            "bb": bbv,
        })
    return in_maps


def _unshard(outs):
    full = np.empty((B, S, F), np.float32)
    for c in range(NCORES):
        b2, qq, _, _ = _core_groups(c)
        full[b2, 256 * qq:256 * (qq + 1), :] = outs[c]
    return full


def _numpy_core(in_map, causal=True):
    """Numpy emulation of the device program (for host-logic validation)."""
    qt = in_map["qt"].astype(np.float32); kt = in_map["kt"].astype(np.float32)
    va = in_map["va"].reshape(S, NG, 33).astype(np.float32)
    wtm = in_map["wt"].astype(np.float32); bbv = in_map["bb"]
    ytv = np.zeros((F, 256), np.float32)
    for g in range(NG):
        sc = kt[g * H:(g + 1) * H].T @ qt[g * H:(g + 1) * H]   # [s2, s1]
        if causal:
            s2i, s1i = np.mgrid[0:S, 0:S]
            sc = np.where(s1i >= s2i, sc, -1e9)
        e = np.exp(sc).astype(np.float16).astype(np.float32)
        if causal:
            e = np.where(s1i >= s2i, e, 0.0).astype(np.float32)
        xt = va[:, g, :].T @ e                                  # [33, s1]
        recip = (1.0 / xt[32]).astype(np.float16).astype(np.float32)
        xs = (xt[0:16] * recip[None, :]).astype(np.float16).astype(np.float32)                  # [h, s1]
        po = 64 * (g % 2)
        for m in range(4):
            for cc in range(2):
                ytv[128 * (g // 2) + po + 16 * m: 128 * (g // 2) + po + 16 * (m + 1),
                    128 * cc:128 * (cc + 1)] = xs[:, 512 * cc + m:512 * (cc + 1):4]
    o = ytv.T @ wtm + bbv[0][None, :]
    return o.astype(np.float32)


def kernel(q, k, v, W_out, b_out, apply_mask, _mock=False):
    q = np.asarray(q, np.float32)
    k = np.asarray(k, np.float32)
    v = np.asarray(v, np.float32)
    W_out = np.asarray(W_out, np.float32)
    b_out = np.asarray(b_out, np.float32)
    causal = bool(int(np.asarray(apply_mask)))
    in_maps = _shard_inputs(q, k, v, W_out, b_out)
    if _mock:
        outs = [_numpy_core(m, causal) for m in in_maps]
        return _unshard(outs)
    from concourse.bass_utils import run_bass_kernel_spmd
    nc = _get_nc(causal)
    res = run_bass_kernel_spmd(nc, in_maps, core_ids=list(range(NCORES)))
    return _unshard([r["o"] for r in res.results])
